# revision 1
# baseline (speedup 1.0000x reference)
"""Trainium2 Bass kernel for nn_Attention_49168785605257.

Causal multi-head self-attention: B=2, N=4096, DIM=512, H=8, DH=64.
Reference applies dim_head**-0.5 scaling TWICE (folded here into Wq as 1/64).

Sharding: one head per NeuronCore (8 cores). Each core computes its head's
attention for BOTH batches (packed into partition halves 0-63 / 64-127) and
its partial output projection o_h = attn_out_h @ Wo[64h:64h+64, :].  The host
sums the 8 partials and adds the bias.

Device-side formulation (per core):
  - All tensors carried transposed ([feature, token]) so the contraction dim
    sits on SBUF partitions; the host pre-transposes x.
  - Flash-attention in S^T orientation: S^T[j,i] tiles come straight out of
    the PE; exp on ScalarE (PSUM->SBUF, [128,1024] groups covering both
    batches); causal masking by multiplying the exp output of the 4 diagonal
    j-blocks per i-chunk with precomputed 0/1 masks; A@V accumulated in PSUM
    with v augmented by a ones-column so row 64 collects the softmax
    denominators; 1/den via Ln then Exp(-x) (one ACT table set); denominator
    broadcast across partitions on GPSIMD; normalize late (softmax linearity).
"""

import os
import sys
from contextlib import ExitStack

import numpy as np

for _p in ("/opt/trn_rl_repo", "/root/.axon_site/_ro/trn_rl_repo"):
    if _p not in sys.path and os.path.isdir(_p):
        sys.path.append(_p)

import ml_dtypes  # noqa: E402

B, N, DIM, H, DH = 2, 4096, 512, 8, 64
N_CORES = 8
CH = 512            # i-chunk width (tokens)
JB = 128            # j-block width (tokens)

BF16 = "bfloat16"
F32 = "float32"


def _pin_act_tables():
    """Make Exp and Ln resolve only to the natural_log_exp set so the kernel
    never swaps ACT table sets (each swap costs ~2.7us and we interleave
    exp-softmax with ln/exp reciprocals). Best-effort: on any surprise just
    leave the default table selection in place (slower, still correct)."""
    try:
        import concourse.bacc as bacc
        import concourse.hw_specs as hw_specs
        import concourse.mybir as mybir
        orig = hw_specs.get_activation_tables

        def patched(module_arch):
            try:
                tabs = dict(orig(module_arch))
                both = {mybir.ActivationFunctionType.Exp, mybir.ActivationFunctionType.Ln}
                target = None
                for name, funcs in tabs.items():
                    if both <= funcs:
                        target = name
                        break
                if target is None:
                    return tabs
                out = {}
                for name, funcs in tabs.items():
                    out[name] = set(funcs) if name == target else set(funcs) - both
                return out
            except Exception:
                return orig(module_arch)

        bacc.get_activation_tables = patched
    except Exception:
        pass


def build_attention_kernel(nc, NB: int):
    """Emit the per-core program. NB = tokens per batch (4096 full size)."""
    import concourse.mybir as mybir
    import concourse.tile as tile
    _pin_act_tables()

    bf16 = mybir.dt.bfloat16
    f32 = mybir.dt.float32
    mult = mybir.AluOpType.mult
    Exp = mybir.ActivationFunctionType.Exp
    Ln = mybir.ActivationFunctionType.Ln

    NCH = NB // CH          # i-chunks per batch
    JTB = NB // JB          # j-blocks per batch

    xT_d = nc.dram_tensor("xT", [DIM, 2 * NB], bf16, kind="ExternalInput").ap()
    wq_d = nc.dram_tensor("wq", [128, 4 * DH], bf16, kind="ExternalInput").ap()
    wk_d = nc.dram_tensor("wk", [128, 4 * DH], bf16, kind="ExternalInput").ap()
    wv_d = nc.dram_tensor("wv", [128, 4 * DH], bf16, kind="ExternalInput").ap()
    wo_d = nc.dram_tensor("wo", [DH, DIM], bf16, kind="ExternalInput").ap()
    mask_d = nc.dram_tensor("masks", [128, 4096], bf16, kind="ExternalInput").ap()
    idup_d = nc.dram_tensor("identup", [128, DH], bf16, kind="ExternalInput").ap()
    oT_d = nc.dram_tensor("oT", [DIM, 2 * NB], f32, kind="ExternalOutput").ap()

    with tile.TileContext(nc) as tc, ExitStack() as ctx:
        const = ctx.enter_context(tc.tile_pool(name="const", bufs=1))
        xpool = ctx.enter_context(tc.tile_pool(name="xp", bufs=16))
        big = ctx.enter_context(tc.tile_pool(name="big", bufs=1))
        ptp = ctx.enter_context(tc.tile_pool(name="ptp", bufs=6))
        rp = ctx.enter_context(tc.tile_pool(name="rp", bufs=3))
        op_sb_pool = ctx.enter_context(tc.tile_pool(name="osb", bufs=4))
        ps_pool = ctx.enter_context(tc.tile_pool(name="ps", bufs=2, space="PSUM"))
        av_pool = ctx.enter_context(tc.tile_pool(name="av", bufs=1, space="PSUM"))
        pv_pool = ctx.enter_context(tc.tile_pool(name="pv", bufs=2, space="PSUM"))

        # ---- weights first so chunk-0 projections can start ASAP ----
        wq_sb = const.tile([128, 4 * DH], bf16, tag="wq")
        wk_sb = const.tile([128, 4 * DH], bf16, tag="wk")
        wv_sb = const.tile([128, 4 * DH], bf16, tag="wv")
        nc.sync.dma_start(wq_sb[:], wq_d[:, :])
        nc.sync.dma_start(wk_sb[:], wk_d[:, :])
        nc.sync.dma_start(wv_sb[:], wv_d[:, :])
        wo_sb = const.tile([DH, DIM], bf16, tag="wo")
        mask_sb = const.tile([128, 4096], bf16, tag="mask")
        idup_sb = const.tile([128, DH], bf16, tag="idup")

        # ---- persistent activations (partition halves: rows 0-63 batch0, 64-127 batch1) ----
        qT = big.tile([128, NB], bf16, tag="qT")
        kT = big.tile([128, NB], bf16, tag="kT")
        vT = big.tile([128, NB], bf16, tag="vT")
        vaug = [big.tile([128, 65 * JTB], bf16, tag=f"vaug{b}", name=f"vaug{b}")
                for b in range(2)]

        xts_pend = {}

        def emit_xt(c):
            """Issue the x-chunk DMAs for chunk c (early, to dodge sync-queue
            head-of-line blocking behind epilogue output DMAs)."""
            xts = []
            for d in range(4):
                xt = xpool.tile([128, 1024], bf16, tag="xt", name=f"xt{c}_{d}")
                nc.gpsimd.dma_start(xt[:], xT_d[128 * d:128 * (d + 1), 1024 * c:1024 * (c + 1)])
                xts.append(xt)
            xts_pend[c] = xts

        def emit_chunk_prep(c):
            """q/k/v projections + v transposes for chunk c."""
            i0 = CH * c
            xts = xts_pend.pop(c)
            if c == 0:
                nc.sync.dma_start(idup_sb[:], idup_d[:, :])
            for w_sb, dst in ((wq_sb, qT), (wk_sb, kT), (wv_sb, vT)):
                ps = pv_pool.tile([128, CH], f32, tag="pv")
                for d in range(4):
                    nc.tensor.matmul(ps[0:64, :], w_sb[:, d * DH:(d + 1) * DH], xts[d][:, 0:512],
                                     start=(d == 0), stop=(d == 3), tile_position=(0, 0),
                                     skip_group_check=True)
                    nc.tensor.matmul(ps[64:128, :], w_sb[:, d * DH:(d + 1) * DH], xts[d][:, 512:1024],
                                     start=(d == 0), stop=(d == 3), tile_position=(0, 64),
                                     skip_group_check=True)
                nc.vector.tensor_copy(dst[:, i0:i0 + CH], ps[:, :])
            if c == 0:
                nc.sync.dma_start(mask_sb[:], mask_d[:, :])
                nc.sync.dma_start(wo_sb[:], wo_d[:, :])
            for tt in range(4 * c, 4 * c + 4):
                pst0 = pv_pool.tile([128, 64], bf16, tag="pv", name="pst0")
                pst1 = pv_pool.tile([128, 64], bf16, tag="pv", name="pst1")
                nc.tensor.matmul(pst0[:], vT[0:64, JB * tt:JB * (tt + 1)], idup_sb[0:64, :],
                                 is_transpose=True, tile_position=(0, 0), skip_group_check=True)
                nc.tensor.matmul(pst1[:], vT[64:128, JB * tt:JB * (tt + 1)], idup_sb[64:128, :],
                                 is_transpose=True, tile_position=(64, 0), skip_group_check=True)
                nc.vector.tensor_copy(vaug[0][:, 65 * tt:65 * tt + 64], pst0[:])
                nc.vector.tensor_copy(vaug[1][:, 65 * tt:65 * tt + 64], pst1[:])

        def emit_epilogue_a2(c, outT_un):
            """1/den chain, off the attention critical path."""
            recip = rp.tile([65, 2048], f32, tag="recip")
            nc.scalar.activation(recip[64:65, 0:1024], outT_un[64:65, 0:1024], Ln)
            nc.scalar.activation(recip[64:65, 1024:2048], recip[64:65, 0:1024], Exp, scale=-1.0)
            riph = rp.tile([1, 1024], f32, tag="riph")
            nc.gpsimd.dma_start(riph[0:1, :], recip[64:65, 1024:2048])
            recipb = rp.tile([64, 1024], f32, tag="recipb")
            nc.gpsimd.partition_broadcast(recipb[0:64, :], riph[0:1, :], channels=64)
            return recipb

        def emit_epilogue_b(c, outT_un, recipb):
            """Deferred per-chunk tail: normalize and project; stream out."""
            outTn = rp.tile([64, 1024], bf16, tag="outTn")
            nc.vector.tensor_tensor(outTn[:], outT_un[0:64, 0:1024], recipb[:], mult)
            for dblk in range(4):
                o_sb = op_sb_pool.tile([128, 1024], f32, tag="o")
                for b in range(2):
                    opp = pv_pool.tile([128, 512], f32, tag="pv", name=f"opp{b}")
                    nc.tensor.matmul(opp[:], wo_sb[:, 128 * dblk:128 * (dblk + 1)],
                                     outTn[0:64, 512 * b:512 * b + 512],
                                     skip_group_check=True)
                    nc.vector.tensor_copy(o_sb[:, 512 * b:512 * (b + 1)], opp[:])
                nc.sync.dma_start(oT_d[128 * dblk:128 * (dblk + 1), 1024 * c:1024 * (c + 1)],
                                  o_sb[:])

        n_up = min(4, NCH)          # chunks prepped upfront (short early chunks)
        emit_xt(0)
        nc.gpsimd.memset(vaug[0][:], 1.0)
        nc.gpsimd.memset(vaug[1][:], 1.0)
        for cc in range(1, n_up):
            emit_xt(cc)
        for cc in range(n_up):
            emit_chunk_prep(cc)
        if n_up < NCH:
            emit_xt(n_up)
        pending_b = None
        for c in range(NCH):
            i0 = CH * c
            # ---- attention for i-chunk c (prev tail + next prep interleaved) ----
            pso = av_pool.tile([65, 1024], f32, tag="av")
            njb = 4 * (c + 1)
            mid = max(1, njb // 2)
            for jb in range(njb):
                if jb == 2 and pending_b is not None:
                    emit_epilogue_b(*pending_b)
                    pending_b = None
                    if n_up <= c + 2 < NCH:
                        emit_xt(c + 2)
                if jb == mid and n_up <= c + 1 < NCH:
                    emit_chunk_prep(c + 1)
                # diagonal blocks: only i-columns >= 128t are causally valid
                t = jb - 4 * c
                off = 128 * t if t > 0 else 0
                w = CH - off
                pss = ps_pool.tile([128, 1024], f32, tag="s")
                nc.tensor.matmul(pss[:, off:512], kT[0:64, JB * jb:JB * (jb + 1)],
                                 qT[0:64, i0 + off:i0 + CH],
                                 start=True, stop=True, tile_position=(0, 0), skip_group_check=True)
                nc.tensor.matmul(pss[:, 512 + off:1024], kT[64:128, JB * jb:JB * (jb + 1)],
                                 qT[64:128, i0 + off:i0 + CH],
                                 start=True, stop=True, tile_position=(64, 0), skip_group_check=True)
                pt = ptp.tile([128, 1024], bf16, tag="pt")
                if off:
                    sub = lambda ap, base=0: ap.rearrange("p (h w) -> p h w", h=2)[:, :, off:]
                    nc.scalar.activation(sub(pt[:]), sub(pss[:]), Exp)
                    if t >= 0:
                        nc.vector.tensor_tensor(
                            sub(pt[:]), sub(pt[:]),
                            sub(mask_sb[:, 1024 * t:1024 * (t + 1)]), mult)
                else:
                    nc.scalar.activation(pt[:], pss[:], Exp)
                    if t == 0:
                        nc.vector.tensor_tensor(pt[:], pt[:], mask_sb[:, 0:1024], mult)
                nc.tensor.matmul(pso[:, off:512], vaug[0][:, 65 * jb:65 * jb + 65], pt[:, off:512],
                                 start=(jb == 0), stop=(jb == njb - 1), skip_group_check=True)
                nc.tensor.matmul(pso[:, 512 + off:1024], vaug[1][:, 65 * jb:65 * jb + 65],
                                 pt[:, 512 + off:1024],
                                 start=(jb == 0), stop=(jb == njb - 1), skip_group_check=True)

            # ---- epilogue part A: evacuate pso in one fp32 copy ----
            outT_un = rp.tile([65, 1024], f32, tag="outT_un")
            nc.vector.tensor_copy(outT_un[:], pso[0:65, 0:1024])
            if (c + 2) < NCH and (c + 2) not in xts_pend and n_up <= c + 2:
                emit_xt(c + 2)   # fallback if the jb==2 site did not fire
            pending_b = (c, outT_un, emit_epilogue_a2(c, outT_un))
        if pending_b is not None:
            emit_epilogue_b(*pending_b)
    return nc


def make_host_constants(NB: int):
    """Masks for the 4 diagonal j-block offsets and the stacked identity."""
    jj = np.arange(JB)[:, None]
    ii = np.arange(CH)[None, :]
    masks = np.zeros((128, 4096), np.float32)            # SBUF layout: mask t at cols 1024t
    for t in range(4):
        m = (ii >= jj + JB * t).astype(np.float32)       # [128, 512]
        masks[:, 1024 * t:1024 * (t + 1)] = np.concatenate([m, m], axis=1)
    identup = np.concatenate([np.eye(DH, dtype=np.float32)] * 2, axis=0)  # [128, 64]
    return (masks.astype(ml_dtypes.bfloat16), identup.astype(ml_dtypes.bfloat16))


_CACHE = {}


def _get_compiled(NB: int):
    key = ("nc", NB)
    if key not in _CACHE:
        import concourse.bacc as bacc
        nc = bacc.Bacc("TRN2", debug=False, num_devices=N_CORES)
        build_attention_kernel(nc, NB)
        nc.compile()
        _CACHE[key] = nc
    return _CACHE[key]


def make_in_maps(x, Wq, Wkv, Wo, NB: int):
    bf = ml_dtypes.bfloat16
    NB = x.shape[1]
    nb_total = x.shape[0] * NB
    xT = x.reshape(nb_total, DIM).T            # [512, B*NB], batch-major cols
    xT = xT.reshape(DIM, 2, NB // CH, CH).transpose(0, 2, 1, 3).reshape(DIM, nb_total)
    xT = np.ascontiguousarray(xT).astype(bf)   # chunk-paired: col = 1024c + 512b + i
    masks, identup = make_host_constants(NB)
    in_maps = []
    def wpack(w):        # [512, 64] -> SBUF layout [128, 256] (d-tile on free dim)
        return np.ascontiguousarray(
            w.reshape(4, 128, DH).transpose(1, 0, 2).reshape(128, 4 * DH)).astype(bf)

    for h in range(N_CORES):
        s = slice(DH * h, DH * (h + 1))
        in_maps.append({
            "xT": xT,
            "wq": wpack(Wq[:, s] / 64.0),
            "wk": wpack(Wkv[:, DH * h:DH * (h + 1)]),
            "wv": wpack(Wkv[:, DIM + DH * h:DIM + DH * (h + 1)]),
            "wo": np.ascontiguousarray(Wo[s, :]).astype(bf),
            "masks": masks,
            "identup": identup,
        })
    return in_maps


def kernel(x, Wq, Wkv, Wo, bo, _run_kwargs=None):
    from concourse.bass_utils import run_bass_kernel_spmd
    x = np.asarray(x, np.float32)
    NB = x.shape[1]
    nc = _get_compiled(NB)
    in_maps = make_in_maps(np.asarray(x), np.asarray(Wq), np.asarray(Wkv), np.asarray(Wo), NB)
    res = run_bass_kernel_spmd(nc, in_maps, core_ids=list(range(N_CORES)),
                               **(_run_kwargs or {}))
    oT = np.zeros((DIM, x.shape[0] * NB), np.float64)
    for c in range(N_CORES):
        oT += res.results[c]["oT"].astype(np.float64)
    # invert chunk-paired layout: col = 1024c + 512b + i  ->  [b, n, D]
    out = (oT.reshape(DIM, NB // CH, 2, CH).transpose(2, 1, 3, 0)
           .reshape(x.shape[0], NB, DIM).astype(np.float32) + np.asarray(bo, np.float32))
    if _run_kwargs is not None:
        _CACHE["last_results"] = res
    return out



# revision 2
# speedup vs baseline: 1.2336x; 1.2336x over previous
"""Trainium2 Bass kernel for nn_Attention_49168785605257.

Causal multi-head self-attention: B=2, N=4096, DIM=512, H=8, DH=64.
Reference applies dim_head**-0.5 scaling TWICE (folded here into Wq as 1/64).

Sharding: one head per NeuronCore (8 cores). Each core computes its head's
attention for BOTH batches (packed into partition halves 0-63 / 64-127) and
its partial output projection o_h = attn_out_h @ Wo[64h:64h+64, :].  The
softmax NORMALIZATION happens on the host: each core ships the unnormalized
projected partial (bf16) plus the per-token softmax denominators; the host
divides and sums the 8 partials, then adds the bias.

Device-side formulation (per core):
  - All tensors carried transposed ([feature, token]) so the contraction dim
    sits on SBUF partitions; the host pre-transposes x.
  - Flash-attention in S^T orientation: S^T[j,i] tiles come straight out of
    the PE; exp on ScalarE (PSUM->SBUF, [128,1024] groups covering both
    batches); causal masking by multiplying the exp output of the 4 diagonal
    j-blocks per i-chunk with precomputed 0/1 masks; A@V accumulated in PSUM
    with v augmented by a ones-column so row 64 collects the softmax
    denominators (shipped to the host, never inverted on device).
  - The j-block loop is software-pipelined: S(jb+1) is issued to the PE
    queue BEFORE A@V(jb), so the in-order PE queue never head-of-line
    blocks on exp(jb) (ScalarE is the steady-state bottleneck engine).
"""

import os
import sys
from contextlib import ExitStack

import numpy as np

for _p in ("/opt/trn_rl_repo", "/root/.axon_site/_ro/trn_rl_repo"):
    if _p not in sys.path and os.path.isdir(_p):
        sys.path.append(_p)

import ml_dtypes  # noqa: E402

B, N, DIM, H, DH = 2, 4096, 512, 8, 64
N_CORES = 8
CH = 512            # i-chunk width (tokens)
JB = 128            # j-block width (tokens)

BF16 = "bfloat16"
F32 = "float32"


def build_attention_kernel(nc, NB: int):
    """Emit the per-core program. NB = tokens per batch (4096 full size)."""
    import concourse.mybir as mybir
    import concourse.tile as tile

    bf16 = mybir.dt.bfloat16
    f32 = mybir.dt.float32
    mult = mybir.AluOpType.mult
    Exp = mybir.ActivationFunctionType.Exp

    NCH = NB // CH          # i-chunks per batch
    JTB = NB // JB          # j-blocks per batch

    xT_d = nc.dram_tensor("xT", [DIM, 2 * NB], bf16, kind="ExternalInput").ap()
    wq_d = nc.dram_tensor("wq", [128, 4 * DH], bf16, kind="ExternalInput").ap()
    wk_d = nc.dram_tensor("wk", [128, 4 * DH], bf16, kind="ExternalInput").ap()
    wv_d = nc.dram_tensor("wv", [128, 4 * DH], bf16, kind="ExternalInput").ap()
    wo_d = nc.dram_tensor("wo", [DH, DIM], bf16, kind="ExternalInput").ap()
    mask_d = nc.dram_tensor("masks", [128, 4096], bf16, kind="ExternalInput").ap()
    idup_d = nc.dram_tensor("identup", [128, DH], bf16, kind="ExternalInput").ap()
    oT_d = nc.dram_tensor("oT", [DIM, 2 * NB], bf16, kind="ExternalOutput").ap()
    den_d = nc.dram_tensor("den", [1, 2 * NB], bf16, kind="ExternalOutput").ap()

    with tile.TileContext(nc) as tc, ExitStack() as ctx:
        const = ctx.enter_context(tc.tile_pool(name="const", bufs=1))
        xpool = ctx.enter_context(tc.tile_pool(name="xp", bufs=16))
        big = ctx.enter_context(tc.tile_pool(name="big", bufs=1))
        ptp = ctx.enter_context(tc.tile_pool(name="ptp", bufs=6))
        rp = ctx.enter_context(tc.tile_pool(name="rp", bufs=2))
        op_sb_pool = ctx.enter_context(tc.tile_pool(name="osb", bufs=4))
        ps_pool = ctx.enter_context(tc.tile_pool(name="ps", bufs=2, space="PSUM"))
        av_pool = ctx.enter_context(tc.tile_pool(name="av", bufs=1, space="PSUM"))
        pv_pool = ctx.enter_context(tc.tile_pool(name="pv", bufs=2, space="PSUM"))

        # ---- weights first so chunk-0 projections can start ASAP ----
        wq_sb = const.tile([128, 4 * DH], bf16, tag="wq")
        wk_sb = const.tile([128, 4 * DH], bf16, tag="wk")
        wv_sb = const.tile([128, 4 * DH], bf16, tag="wv")
        nc.sync.dma_start(wq_sb[:], wq_d[:, :])
        nc.sync.dma_start(wk_sb[:], wk_d[:, :])
        nc.sync.dma_start(wv_sb[:], wv_d[:, :])
        wo_sb = const.tile([DH, DIM], bf16, tag="wo")
        mask_sb = const.tile([128, 4096], bf16, tag="mask")
        idup_sb = const.tile([128, DH], bf16, tag="idup")

        # ---- persistent activations (partition halves: rows 0-63 batch0, 64-127 batch1) ----
        qT = big.tile([128, NB], bf16, tag="qT")
        kT = big.tile([128, NB], bf16, tag="kT")
        vT = big.tile([128, NB], bf16, tag="vT")
        vaug = [big.tile([128, 65 * JTB], bf16, tag=f"vaug{b}", name=f"vaug{b}")
                for b in range(2)]

        xts_pend = {}
        xts_done = set()

        def emit_xt(c):
            """Issue the x-chunk DMAs for chunk c (gpsimd queue carries only
            these, so they never block anything latency-critical)."""
            xts = []
            for d in range(4):
                xt = xpool.tile([128, 1024], bf16, tag="xt", name=f"xt{c}_{d}")
                nc.gpsimd.dma_start(xt[:], xT_d[128 * d:128 * (d + 1), 1024 * c:1024 * (c + 1)])
                xts.append(xt)
            xts_pend[c] = xts
            xts_done.add(c)

        def emit_chunk_prep(c):
            """q/k/v projections + v transposes for chunk c."""
            i0 = CH * c
            xts = xts_pend.pop(c)
            if c == 0:
                nc.sync.dma_start(idup_sb[:], idup_d[:, :])
            for w_sb, dst in ((wq_sb, qT), (wk_sb, kT), (wv_sb, vT)):
                ps = pv_pool.tile([128, CH], f32, tag="pv")
                for d in range(4):
                    nc.tensor.matmul(ps[0:64, :], w_sb[:, d * DH:(d + 1) * DH], xts[d][:, 0:512],
                                     start=(d == 0), stop=(d == 3), tile_position=(0, 0),
                                     skip_group_check=True)
                    nc.tensor.matmul(ps[64:128, :], w_sb[:, d * DH:(d + 1) * DH], xts[d][:, 512:1024],
                                     start=(d == 0), stop=(d == 3), tile_position=(0, 64),
                                     skip_group_check=True)
                nc.vector.tensor_copy(dst[:, i0:i0 + CH], ps[:, :])
            if c == 0:
                nc.sync.dma_start(mask_sb[:], mask_d[:, :])
                nc.sync.dma_start(wo_sb[:], wo_d[:, :])
            for tt in range(4 * c, 4 * c + 4):
                pst0 = pv_pool.tile([128, 64], bf16, tag="pv", name="pst0")
                pst1 = pv_pool.tile([128, 64], bf16, tag="pv", name="pst1")
                nc.tensor.matmul(pst0[:], vT[0:64, JB * tt:JB * (tt + 1)], idup_sb[0:64, :],
                                 is_transpose=True, tile_position=(0, 0), skip_group_check=True)
                nc.tensor.matmul(pst1[:], vT[64:128, JB * tt:JB * (tt + 1)], idup_sb[64:128, :],
                                 is_transpose=True, tile_position=(64, 0), skip_group_check=True)
                nc.vector.tensor_copy(vaug[0][:, 65 * tt:65 * tt + 64], pst0[:])
                nc.vector.tensor_copy(vaug[1][:, 65 * tt:65 * tt + 64], pst1[:])

        def emit_s_exp(c, jb):
            """S^T matmul pair for j-block jb of chunk c, then exp (+ causal
            mask multiply on the 4 diagonal blocks). Returns (pt, off)."""
            i0 = CH * c
            t = jb - 4 * c
            off = 128 * t if t > 0 else 0
            pss = ps_pool.tile([128, 1024], f32, tag="s")
            nc.tensor.matmul(pss[:, off:512], kT[0:64, JB * jb:JB * (jb + 1)],
                             qT[0:64, i0 + off:i0 + CH],
                             start=True, stop=True, tile_position=(0, 0), skip_group_check=True)
            nc.tensor.matmul(pss[:, 512 + off:1024], kT[64:128, JB * jb:JB * (jb + 1)],
                             qT[64:128, i0 + off:i0 + CH],
                             start=True, stop=True, tile_position=(64, 0), skip_group_check=True)
            pt = ptp.tile([128, 1024], bf16, tag="pt")
            if off:
                sub = lambda ap: ap.rearrange("p (h w) -> p h w", h=2)[:, :, off:]
                nc.scalar.activation(sub(pt[:]), sub(pss[:]), Exp)
                nc.vector.tensor_tensor(
                    sub(pt[:]), sub(pt[:]),
                    sub(mask_sb[:, 1024 * t:1024 * (t + 1)]), mult)
            else:
                nc.scalar.activation(pt[:], pss[:], Exp)
                if t == 0:
                    nc.vector.tensor_tensor(pt[:], pt[:], mask_sb[:, 0:1024], mult)
            return pt, off

        def emit_av(jb, pt, off, pso, first, last):
            nc.tensor.matmul(pso[:, off:512], vaug[0][:, 65 * jb:65 * jb + 65], pt[:, off:512],
                             start=first, stop=last, skip_group_check=True)
            nc.tensor.matmul(pso[:, 512 + off:1024], vaug[1][:, 65 * jb:65 * jb + 65],
                             pt[:, 512 + off:1024],
                             start=first, stop=last, skip_group_check=True)

        def emit_outproj(c, outT_un, dblks):
            """Unnormalized output projection for chunk c, dim blocks dblks."""
            for dblk in dblks:
                o_sb = op_sb_pool.tile([128, 1024], bf16, tag="o")
                for b in range(2):
                    opp = pv_pool.tile([128, 512], f32, tag="pv", name=f"opp{b}")
                    nc.tensor.matmul(opp[:], wo_sb[:, 128 * dblk:128 * (dblk + 1)],
                                     outT_un[0:64, 512 * b:512 * b + 512],
                                     skip_group_check=True)
                    nc.vector.tensor_copy(o_sb[:, 512 * b:512 * (b + 1)], opp[:])
                nc.sync.dma_start(oT_d[128 * dblk:128 * (dblk + 1), 1024 * c:1024 * (c + 1)],
                                  o_sb[:])

        n_up = min(2, NCH)          # chunks prepped upfront
        emit_xt(0)
        nc.gpsimd.memset(vaug[0][:], 1.0)
        nc.gpsimd.memset(vaug[1][:], 1.0)
        for cc in range(1, n_up + 1):
            if cc < NCH:
                emit_xt(cc)
        for cc in range(n_up):
            emit_chunk_prep(cc)
        preps_done = n_up
        pending = None              # (c, outT_un) awaiting output projection
        for c in range(NCH):
            njb = 4 * (c + 1)
            mid = njb // 2
            pso = av_pool.tile([65, 1024], f32, tag="av")
            pt_q = {0: emit_s_exp(c, 0)}
            for jb in range(njb):
                if jb + 1 < njb:
                    pt_q[jb + 1] = emit_s_exp(c, jb + 1)
                if jb == 1 and c + 3 < NCH and (c + 3) not in xts_done:
                    emit_xt(c + 3)
                if jb == 3 and pending is not None:
                    emit_outproj(pending[0], pending[1], (0, 1))
                if jb == 5 and pending is not None:
                    emit_outproj(pending[0], pending[1], (2, 3))
                    pending = None
                if jb == mid and preps_done < NCH and preps_done <= c + 2:
                    emit_chunk_prep(preps_done)
                    preps_done += 1
                pt, off = pt_q.pop(jb)
                emit_av(jb, pt, off, pso, first=(jb == 0), last=(jb == njb - 1))
            # evacuate pso (one bf16 cast; row 64 holds the denominators)
            outT_un = rp.tile([65, 1024], bf16, tag="outT_un")
            nc.vector.tensor_copy(outT_un[:], pso[0:65, 0:1024])
            nc.sync.dma_start(den_d[0:1, 1024 * c:1024 * (c + 1)], outT_un[64:65, :])
            if pending is not None:      # chunk 0 (njb=4) can't flush in-loop
                emit_outproj(pending[0], pending[1], (0, 1, 2, 3))
            pending = (c, outT_un)
        if pending is not None:
            emit_outproj(pending[0], pending[1], (0, 1, 2, 3))
    return nc


def make_host_constants(NB: int):
    """Masks for the 4 diagonal j-block offsets and the stacked identity."""
    jj = np.arange(JB)[:, None]
    ii = np.arange(CH)[None, :]
    masks = np.zeros((128, 4096), np.float32)            # SBUF layout: mask t at cols 1024t
    for t in range(4):
        m = (ii >= jj + JB * t).astype(np.float32)       # [128, 512]
        masks[:, 1024 * t:1024 * (t + 1)] = np.concatenate([m, m], axis=1)
    identup = np.concatenate([np.eye(DH, dtype=np.float32)] * 2, axis=0)  # [128, 64]
    return (masks.astype(ml_dtypes.bfloat16), identup.astype(ml_dtypes.bfloat16))


_CACHE = {}


def _get_compiled(NB: int):
    key = ("nc", NB)
    if key not in _CACHE:
        import concourse.bacc as bacc
        nc = bacc.Bacc("TRN2", debug=False, num_devices=N_CORES)
        build_attention_kernel(nc, NB)
        nc.compile()
        _CACHE[key] = nc
    return _CACHE[key]


def make_in_maps(x, Wq, Wkv, Wo, NB: int):
    bf = ml_dtypes.bfloat16
    NB = x.shape[1]
    nb_total = x.shape[0] * NB
    xT = x.reshape(nb_total, DIM).T            # [512, B*NB], batch-major cols
    xT = xT.reshape(DIM, 2, NB // CH, CH).transpose(0, 2, 1, 3).reshape(DIM, nb_total)
    xT = np.ascontiguousarray(xT).astype(bf)   # chunk-paired: col = 1024c + 512b + i
    masks, identup = make_host_constants(NB)
    in_maps = []
    def wpack(w):        # [512, 64] -> SBUF layout [128, 256] (d-tile on free dim)
        return np.ascontiguousarray(
            w.reshape(4, 128, DH).transpose(1, 0, 2).reshape(128, 4 * DH)).astype(bf)

    for h in range(N_CORES):
        s = slice(DH * h, DH * (h + 1))
        in_maps.append({
            "xT": xT,
            "wq": wpack(Wq[:, s] / 64.0),
            "wk": wpack(Wkv[:, DH * h:DH * (h + 1)]),
            "wv": wpack(Wkv[:, DIM + DH * h:DIM + DH * (h + 1)]),
            "wo": np.ascontiguousarray(Wo[s, :]).astype(bf),
            "masks": masks,
            "identup": identup,
        })
    return in_maps


def kernel(x, Wq, Wkv, Wo, bo, _run_kwargs=None):
    from concourse.bass_utils import run_bass_kernel_spmd
    x = np.asarray(x, np.float32)
    NB = x.shape[1]
    nc = _get_compiled(NB)
    in_maps = make_in_maps(np.asarray(x), np.asarray(Wq), np.asarray(Wkv), np.asarray(Wo), NB)
    res = run_bass_kernel_spmd(nc, in_maps, core_ids=list(range(N_CORES)),
                               **(_run_kwargs or {}))
    oT = np.zeros((DIM, x.shape[0] * NB), np.float64)
    for c in range(N_CORES):
        den = res.results[c]["den"].astype(np.float64)          # [1, B*NB]
        oT += res.results[c]["oT"].astype(np.float64) / den
    # invert chunk-paired layout: col = 1024c + 512b + i  ->  [b, n, D]
    out = (oT.reshape(DIM, NB // CH, 2, CH).transpose(2, 1, 3, 0)
           .reshape(x.shape[0], NB, DIM).astype(np.float32) + np.asarray(bo, np.float32))
    if _run_kwargs is not None:
        _CACHE["last_results"] = res
    return out
